# revision 4
# baseline (speedup 1.0000x reference)
"""Trainium2 Bass kernel for ACMELayerWithTag:
    out = x @ (dequant(weight_base) + T_tag)^T + bias

Full shapes: x [8192, 4096] f32, weight_base [4096, 4096] int32 (ternary),
T_tag [4096, 4096] f32, bias [4096] f32 -> out [8192, 4096] f32.

Strategy: data-parallel across the 8 NeuronCores -- each core owns 1024
rows of x and computes its 1024x4096 slice of the output. Weights are
replicated (each core streams the full W once). No collectives.

Per-core device kernel (identical SPMD graph):
  - inputs are passed pre-transposed (layout-only host transform) so both
    matmul operands have the contraction dim (IN) on SBUF partitions:
      xt   [4096, 1024] f32  = x_shard^T
      wbt  [4096, 4096] int8 = weight_base^T   (lossless ternary re-encode)
      tagt [4096, 4096] f32  = T_tag^T
      bias [4096] f32
  - device does dequant+add (DVE: int8 + f32 -> bf16), x cast f32->bf16,
    then 8 o-blocks x 8 n-tiles x 32 k-steps of 128x128x512 bf16 matmuls
    accumulating in f32 PSUM; bias is partition-broadcast once and fused
    into the PSUM->SBUF copy.
"""

import sys

if "/opt/trn_rl_repo" not in sys.path:
    sys.path.insert(0, "/opt/trn_rl_repo")

import numpy as np

N_FULL, D_IN, D_OUT = 8192, 4096, 4096
N_CORES = 8
N_SHARD = N_FULL // N_CORES

P = 128          # SBUF partitions / matmul tile edge
OBW = 512        # o-block width (PSUM bank = 512 f32)
KCS = 4          # k-subtiles staged per weight chunk


def build_core_graph(n_shard=N_SHARD, d_in=D_IN, d_out=D_OUT, chunked=True):
    """Build the per-core Bacc graph. Returns the compiled Bacc object.

    chunked=True allocates x^T / W_eff^T as per-chunk tiles so matmuls can
    start as soon as the first chunks land (fine-grained deps) instead of
    waiting for whole-tensor staging.
    """
    import concourse.mybir as mybir
    import concourse.tile as tile
    from concourse import bacc

    KO = d_in // P        # k-subtiles (contraction steps of 128)
    NT = n_shard // P     # n-tiles of 128 rows
    OB = d_out // OBW     # o-blocks of 512 output channels
    kcs = KCS
    while KO % kcs:
        kcs -= 1
    KC = KO // kcs        # weight chunks per o-block

    nc = bacc.Bacc("TRN2", target_bir_lowering=False, num_devices=N_CORES)

    xt = nc.dram_tensor("xt", [d_in, n_shard], mybir.dt.float32, kind="ExternalInput").ap()
    wbt = nc.dram_tensor("wbt", [d_in, d_out], mybir.dt.int8, kind="ExternalInput").ap()
    tagt = nc.dram_tensor("tagt", [d_in, d_out], mybir.dt.float32, kind="ExternalInput").ap()
    bias = nc.dram_tensor("bias", [d_out], mybir.dt.float32, kind="ExternalInput").ap()
    out = nc.dram_tensor("out", [n_shard, d_out], mybir.dt.float32, kind="ExternalOutput").ap()

    # stride-permuted views: contraction dim on partitions
    xt_r = xt.rearrange("(c p) n -> c p n", p=P)        # [KO, 128, n_shard]
    wbt_r = wbt.rearrange("(c p) o -> p c o", p=P)      # [128, KO, d_out]
    tagt_r = tagt.rearrange("(c p) o -> p c o", p=P)
    out_r = out.rearrange("(t p) o -> t p o", p=P)      # [NT, 128, d_out]

    with tile.TileContext(nc) as tc:
        with (
            tc.tile_pool(name="xpers", bufs=1) as xpers,
            tc.tile_pool(name="xstage", bufs=2) as xstage,
            tc.tile_pool(name="weff", bufs=2) as weffp,
            tc.tile_pool(name="wstage", bufs=2) as wstage,
            tc.tile_pool(name="biasp", bufs=1) as biasp,
            tc.tile_pool(name="outp", bufs=4) as outp,
            tc.tile_pool(name="psum", bufs=2, space="PSUM") as psump,
        ):
            # bias: load to partition 0, broadcast to all partitions
            bias_row = biasp.tile([1, d_out], mybir.dt.float32)
            bias_bc = biasp.tile([P, d_out], mybir.dt.float32)
            nc.sync.dma_start(bias_row[:], bias[None, :])
            nc.gpsimd.partition_broadcast(bias_bc[:], bias_row[0:1, :])

            # x^T: stream in f32, cast to bf16, keep resident
            if chunked:
                xbf_t = []
                for ko in range(KO):
                    xs = xstage.tile([P, n_shard], mybir.dt.float32)
                    xb = xpers.tile([P, n_shard], mybir.dt.bfloat16, tag=f"xbf{ko}")
                    nc.sync.dma_start(xs[:], xt_r[ko])
                    nc.vector.tensor_copy(xb[:], xs[:])
                    xbf_t.append(xb)
                lhsT = lambda ko, nsl: xbf_t[ko][:, nsl]
            else:
                xbf = xpers.tile([P, KO, n_shard], mybir.dt.bfloat16)
                for ko in range(KO):
                    xs = xstage.tile([P, n_shard], mybir.dt.float32)
                    nc.sync.dma_start(xs[:], xt_r[ko])
                    nc.vector.tensor_copy(xbf[:, ko, :], xs[:])
                lhsT = lambda ko, nsl: xbf[:, ko, nsl]

            for ob in range(OB):
                osl = slice(ob * OBW, (ob + 1) * OBW)
                # stage weight chunks, dequant+add -> bf16 W_eff^T slice
                if chunked:
                    weff_t = []
                    for kc in range(KC):
                        csl = slice(kc * kcs, (kc + 1) * kcs)
                        wbs = wstage.tile([P, kcs, OBW], mybir.dt.int8, tag="wbs")
                        tgs = wstage.tile([P, kcs, OBW], mybir.dt.float32, tag="tgs")
                        wf = weffp.tile([P, kcs, OBW], mybir.dt.bfloat16, tag=f"weff{kc}")
                        nc.sync.dma_start(wbs[:], wbt_r[:, csl, osl])
                        nc.sync.dma_start(tgs[:], tagt_r[:, csl, osl])
                        nc.vector.tensor_tensor(wf[:], wbs[:], tgs[:], mybir.AluOpType.add)
                        weff_t.append(wf)
                    rhs = lambda ko: weff_t[ko // kcs][:, ko % kcs, :]
                else:
                    weff = weffp.tile([P, KO, OBW], mybir.dt.bfloat16)
                    for kc in range(KC):
                        csl = slice(kc * kcs, (kc + 1) * kcs)
                        wbs = wstage.tile([P, kcs, OBW], mybir.dt.int8, tag="wbs")
                        tgs = wstage.tile([P, kcs, OBW], mybir.dt.float32, tag="tgs")
                        nc.sync.dma_start(wbs[:], wbt_r[:, csl, osl])
                        nc.sync.dma_start(tgs[:], tagt_r[:, csl, osl])
                        nc.vector.tensor_tensor(
                            weff[:, csl, :], wbs[:], tgs[:], mybir.AluOpType.add
                        )
                    rhs = lambda ko: weff[:, ko, :]
                # matmuls: psum[n, o] = sum_k xbf[k, n] * weff[k, o]
                for nt in range(NT):
                    nsl = slice(nt * P, (nt + 1) * P)
                    ps = psump.tile([P, OBW], mybir.dt.float32)
                    for ko in range(KO):
                        nc.tensor.matmul(
                            ps[:],
                            lhsT=lhsT(ko, nsl),
                            rhs=rhs(ko),
                            start=(ko == 0),
                            stop=(ko == KO - 1),
                        )
                    osb = outp.tile([P, OBW], mybir.dt.float32)
                    nc.vector.tensor_tensor(
                        osb[:], ps[:], bias_bc[:, osl], mybir.AluOpType.add
                    )
                    nc.sync.dma_start(out_r[nt, :, osl], osb[:])

    nc.compile()
    return nc


def shard_inputs(x, weight_base, T_tag, bias):
    """Host-side layout transforms + sharding. Returns in_maps for 8 cores."""
    wbt = np.ascontiguousarray(weight_base.T).astype(np.int8)
    tagt = np.ascontiguousarray(T_tag.T.astype(np.float32))
    bias = np.ascontiguousarray(bias.astype(np.float32))
    in_maps = []
    for c in range(N_CORES):
        xs = np.ascontiguousarray(x[c * N_SHARD:(c + 1) * N_SHARD, :].T.astype(np.float32))
        in_maps.append({"xt": xs, "wbt": wbt, "tagt": tagt, "bias": bias})
    return in_maps


_CACHE = {}


def _get_nc():
    if "nc" not in _CACHE:
        _CACHE["nc"] = build_core_graph()
    return _CACHE["nc"]


def get_runtime():
    """Build (once) and return the SPMD runtime: a cached jitted callable that
    executes the per-core NEFF on the 8 axon NeuronCores via shard_map.
    Mirrors concourse.bass2jax.run_bass_via_pjrt, but keeps the jitted
    function so repeat calls don't re-trace/re-compile."""
    if "rt" in _CACHE:
        return _CACHE["rt"]

    import jax
    import concourse.mybir as mybir
    from jax.experimental.shard_map import shard_map
    from jax.sharding import Mesh, PartitionSpec
    from concourse.bass2jax import (
        _bass_exec_p,
        install_neuronx_cc_hook,
        partition_id_tensor,
    )

    nc = _get_nc()
    install_neuronx_cc_hook()
    assert nc.dbg_addr is None
    partition_name = nc.partition_id_tensor.name if nc.partition_id_tensor else None

    in_names, out_names, out_avals = [], [], []
    for alloc in nc.m.functions[0].allocations:
        if not isinstance(alloc, mybir.MemoryLocationSet):
            continue
        name = alloc.memorylocations[0].name
        if alloc.kind == "ExternalInput":
            if name != partition_name:
                in_names.append(name)
        elif alloc.kind == "ExternalOutput":
            out_names.append(name)
            out_avals.append(
                jax.core.ShapedArray(tuple(alloc.tensor_shape), mybir.dt.np(alloc.dtype))
            )
    n_params = len(in_names)
    all_names = tuple(in_names) + tuple(out_names)
    if partition_name is not None:
        all_names = all_names + (partition_name,)

    def _body(*args):
        operands = list(args)
        if partition_name is not None:
            operands.append(partition_id_tensor())
        outs = _bass_exec_p.bind(
            *operands,
            out_avals=tuple(out_avals),
            in_names=all_names,
            out_names=tuple(out_names),
            lowering_input_output_aliases=(),
            sim_require_finite=True,
            sim_require_nnan=True,
            nc=nc,
        )
        return tuple(outs)

    devices = jax.devices()[:N_CORES]
    assert len(devices) == N_CORES
    mesh = Mesh(np.asarray(devices), ("core",))
    n_outs = len(out_names)
    sharded = jax.jit(
        shard_map(
            _body,
            mesh=mesh,
            in_specs=(PartitionSpec("core"),) * (n_params + n_outs),
            out_specs=(PartitionSpec("core"),) * n_outs,
            check_rep=False,
        ),
        donate_argnums=tuple(range(n_params, n_params + n_outs)),
        keep_unused=True,
    )
    _CACHE["rt"] = {
        "nc": nc,
        "fn": sharded,
        "mesh": mesh,
        "in_names": in_names,
        "out_names": out_names,
        "out_avals": out_avals,
    }
    return _CACHE["rt"]


def concat_inputs(in_maps, rt):
    return [
        np.concatenate([np.asarray(m[name]) for m in in_maps], axis=0)
        for name in rt["in_names"]
    ]


def zero_outputs(rt):
    return [
        np.zeros((N_CORES * a.shape[0], *a.shape[1:]), a.dtype) for a in rt["out_avals"]
    ]


def kernel(x, weight_base, T_tag, bias):
    rt = get_runtime()
    in_maps = shard_inputs(
        np.asarray(x), np.asarray(weight_base), np.asarray(T_tag), np.asarray(bias)
    )
    outs = rt["fn"](*concat_inputs(in_maps, rt), *zero_outputs(rt))
    return np.asarray(outs[rt["out_names"].index("out")])


# revision 28
# speedup vs baseline: 172.0180x; 172.0180x over previous
"""Trainium2 Bass kernel for ACMELayerWithTag:
    out = x @ (dequant(weight_base) + T_tag)^T + bias

Full shapes: x [8192, 4096] f32, weight_base [4096, 4096] int32 (ternary),
T_tag [4096, 4096] f32, bias [4096] f32 -> out [8192, 4096] f32.

Strategy: data-parallel across the 8 NeuronCores -- each core owns 1024
rows of x and computes its 1024x4096 slice of the output. Weights are
replicated (each core streams the full W once). No collectives.

Per-core device kernel (identical SPMD graph):
  - inputs are passed pre-transposed (layout-only host transform) so both
    matmul operands have the contraction dim (IN) on SBUF partitions:
      xt   [4096, 1024] bf16 = x_shard^T in the kernel's compute encoding
                               (bit-identical to casting on device)
      wbt  [4096, 4096] int8 = weight_base^T   (lossless ternary re-encode)
      tagt [4096, 4096] f32  = T_tag^T
      bias [4096] f32
  - device does dequant+add (DVE: int8 + f32 -> bf16 W_eff^T, streamed in
    4-ksubtile chunks), then per 512-wide o-block runs k-outer matmuls:
    8 concurrent PSUM accumulators (one per 128-row n-tile), 32 k-steps of
    [128x128]@[128x512] bf16 with f32 PSUM accumulation, starting as soon
    as the first weight chunk lands; bias is partition-broadcast once and
    fused into the PSUM->SBUF copy; ~98% of the graph's time is TensorE.
  - measured ~0.6 ms/exec on silicon (loop-differential; PE-bound).
"""

import sys

if "/opt/trn_rl_repo" not in sys.path:
    sys.path.insert(0, "/opt/trn_rl_repo")

import numpy as np

N_FULL, D_IN, D_OUT = 8192, 4096, 4096
N_CORES = 8
N_SHARD = N_FULL // N_CORES

P = 128          # SBUF partitions / matmul tile edge
OBW = 512        # o-block width (PSUM bank = 512 f32)
KCS = 4          # k-subtiles staged per weight chunk


def build_core_graph(n_shard=N_SHARD, d_in=D_IN, d_out=D_OUT, chunked=True, repeat=1,
                     xt_bf16=True, korder=True, wstat=False, kcs_val=KCS,
                     wstage_bufs=2, outp_bufs=4):
    """Build the per-core Bacc graph. Returns the compiled Bacc object.

    chunked=True allocates x^T / W_eff^T as per-chunk tiles so matmuls can
    start as soon as the first chunks land (fine-grained deps) instead of
    waiting for whole-tensor staging.

    repeat>1 runs the whole compute sequence N times inside one NEFF
    (idempotent; used to measure on-device time differentially, cancelling
    per-call dispatch/transfer overhead).
    """
    import concourse.mybir as mybir
    import concourse.tile as tile
    from concourse import bacc

    KO = d_in // P        # k-subtiles (contraction steps of 128)
    NT = n_shard // P     # n-tiles of 128 rows
    OB = d_out // OBW     # o-blocks of 512 output channels
    kcs = kcs_val
    while KO % kcs:
        kcs -= 1
    KC = KO // kcs        # weight chunks per o-block

    nc = bacc.Bacc("TRN2", target_bir_lowering=False, num_devices=N_CORES)

    xt_dt = mybir.dt.bfloat16 if xt_bf16 else mybir.dt.float32
    xt = nc.dram_tensor("xt", [d_in, n_shard], xt_dt, kind="ExternalInput").ap()
    wbt = nc.dram_tensor("wbt", [d_in, d_out], mybir.dt.int8, kind="ExternalInput").ap()
    tagt = nc.dram_tensor("tagt", [d_in, d_out], mybir.dt.float32, kind="ExternalInput").ap()
    bias = nc.dram_tensor("bias", [d_out], mybir.dt.float32, kind="ExternalInput").ap()
    # wstat mode writes the transposed product [d_out, n_shard]; the host
    # un-transposes when assembling the full output.
    out_shape = [d_out, n_shard] if wstat else [n_shard, d_out]
    out = nc.dram_tensor("out", out_shape, mybir.dt.float32, kind="ExternalOutput").ap()

    # stride-permuted views: contraction dim on partitions
    xt_r = xt.rearrange("(c p) n -> c p n", p=P)        # [KO, 128, n_shard]
    wbt_r = wbt.rearrange("(c p) o -> p c o", p=P)      # [128, KO, d_out]
    tagt_r = tagt.rearrange("(c p) o -> p c o", p=P)
    out_r = out.rearrange("(t p) o -> t p o", p=P)      # [NT|OT, 128, *]

    with tile.TileContext(nc) as tc:
        with (
            tc.tile_pool(name="xpers", bufs=1) as xpers,
            tc.tile_pool(name="xstage", bufs=2) as xstage,
            tc.tile_pool(name="weff", bufs=2) as weffp,
            tc.tile_pool(name="wstage", bufs=wstage_bufs) as wstage,
            tc.tile_pool(name="biasp", bufs=1) as biasp,
            tc.tile_pool(name="outp", bufs=outp_bufs) as outp,
            tc.tile_pool(name="psum", bufs=1 if korder else 2, space="PSUM") as psump,
        ):
            if wstat:
                # out^T layout: bias varies along partitions; stage it
                # partition-major [128, d_out/128] for per-partition scalar add
                bias_pt = biasp.tile([P, d_out // P], mybir.dt.float32)
                nc.sync.dma_start(bias_pt[:], bias.rearrange("(t p) -> p t", p=P))
            else:
                # bias: load to partition 0, broadcast to all partitions
                bias_row = biasp.tile([1, d_out], mybir.dt.float32)
                bias_bc = biasp.tile([P, d_out], mybir.dt.float32)
                nc.sync.dma_start(bias_row[:], bias[None, :])
                nc.gpsimd.partition_broadcast(bias_bc[:], bias_row[0:1, :])

            def rep_body():
                # x^T: load (bf16 direct, or f32 staged + cast), keep resident
                if xt_bf16:
                    xbf_t = []
                    for ko in range(KO):
                        xb = xpers.tile([P, n_shard], mybir.dt.bfloat16, tag=f"xbf{ko}")
                        nc.sync.dma_start(xb[:], xt_r[ko])
                        xbf_t.append(xb)
                    lhsT = lambda ko, nsl: xbf_t[ko][:, nsl]
                elif chunked:
                    xbf_t = []
                    for ko in range(KO):
                        xs = xstage.tile([P, n_shard], mybir.dt.float32)
                        xb = xpers.tile([P, n_shard], mybir.dt.bfloat16, tag=f"xbf{ko}")
                        nc.sync.dma_start(xs[:], xt_r[ko])
                        nc.vector.tensor_copy(xb[:], xs[:])
                        xbf_t.append(xb)
                    lhsT = lambda ko, nsl: xbf_t[ko][:, nsl]
                else:
                    xbf = xpers.tile([P, KO, n_shard], mybir.dt.bfloat16, tag="xbf")
                    for ko in range(KO):
                        xs = xstage.tile([P, n_shard], mybir.dt.float32)
                        nc.sync.dma_start(xs[:], xt_r[ko])
                        nc.vector.tensor_copy(xbf[:, ko, :], xs[:])
                    lhsT = lambda ko, nsl: xbf[:, ko, nsl]

                for ob in range(OB):
                    osl = slice(ob * OBW, (ob + 1) * OBW)

                    def stage_chunk(kc):
                        csl = slice(kc * kcs, (kc + 1) * kcs)
                        wbs = wstage.tile([P, kcs, OBW], mybir.dt.int8, tag="wbs")
                        tgs = wstage.tile([P, kcs, OBW], mybir.dt.float32, tag="tgs")
                        wf = weffp.tile([P, kcs, OBW], mybir.dt.bfloat16, tag=f"weff{kc}")
                        nc.sync.dma_start(wbs[:], wbt_r[:, csl, osl])
                        nc.sync.dma_start(tgs[:], tagt_r[:, csl, osl])
                        nc.vector.tensor_tensor(wf[:], wbs[:], tgs[:], mybir.AluOpType.add)
                        return wf

                    if wstat:
                        # weights-stationary: lhsT = 128-wide W_eff^T o-slice,
                        # moving operand = x (2 n-halves of 512) -> each
                        # LDWEIGHTS feeds 2 matmuls. Output is [o, n] (psum
                        # partition dim = o); host un-transposes.
                        OS = OBW // P            # stationary o-slices per block
                        MW = min(OBW, n_shard)   # moving-operand width
                        NH = n_shard // MW       # moving n-slices
                        pss = [[psump.tile([P, MW], mybir.dt.float32,
                                           name=f"ps{s}_{h}", tag=f"ps{s}_{h}")
                                for h in range(NH)] for s in range(OS)]
                        for kc in range(KC):
                            wf = stage_chunk(kc)
                            for j in range(kcs):
                                ko = kc * kcs + j
                                for s in range(OS):
                                    lh = wf[:, j, s * P:(s + 1) * P]
                                    for h in range(NH):
                                        nc.tensor.matmul(
                                            pss[s][h][:],
                                            lhsT=lh,
                                            rhs=lhsT(ko, slice(h * MW, (h + 1) * MW)),
                                            start=(ko == 0),
                                            stop=(ko == KO - 1),
                                        )
                        for s in range(OS):
                            ot = ob * OS + s
                            for h in range(NH):
                                osb = outp.tile([P, MW], mybir.dt.float32)
                                nc.vector.tensor_scalar_add(
                                    osb[:], pss[s][h][:], bias_pt[:, ot:ot + 1]
                                )
                                nc.sync.dma_start(
                                    out_r[ot, :, h * MW:(h + 1) * MW], osb[:]
                                )
                    elif korder:
                        # k-outer: 8 concurrent PSUM accumulators, matmuls start
                        # as soon as the first weight chunk lands; chunk kc+1
                        # streams while the PE consumes chunk kc.
                        pss = [psump.tile([P, OBW], mybir.dt.float32,
                                          name=f"ps{nt}", tag=f"ps{nt}")
                               for nt in range(NT)]
                        for kc in range(KC):
                            wf = stage_chunk(kc)
                            for nt in range(NT):
                                nsl = slice(nt * P, (nt + 1) * P)
                                for j in range(kcs):
                                    ko = kc * kcs + j
                                    nc.tensor.matmul(
                                        pss[nt][:],
                                        lhsT=lhsT(ko, nsl),
                                        rhs=wf[:, j, :],
                                        start=(ko == 0),
                                        stop=(ko == KO - 1),
                                    )
                        for nt in range(NT):
                            osb = outp.tile([P, OBW], mybir.dt.float32)
                            nc.vector.tensor_tensor(
                                osb[:], pss[nt][:], bias_bc[:, osl], mybir.AluOpType.add
                            )
                            nc.sync.dma_start(out_r[nt, :, osl], osb[:])
                    else:
                        weff_t = [stage_chunk(kc) for kc in range(KC)]
                        rhs = lambda ko: weff_t[ko // kcs][:, ko % kcs, :]
                        for nt in range(NT):
                            nsl = slice(nt * P, (nt + 1) * P)
                            ps = psump.tile([P, OBW], mybir.dt.float32)
                            for ko in range(KO):
                                nc.tensor.matmul(
                                    ps[:],
                                    lhsT=lhsT(ko, nsl),
                                    rhs=rhs(ko),
                                    start=(ko == 0),
                                    stop=(ko == KO - 1),
                                )
                            osb = outp.tile([P, OBW], mybir.dt.float32)
                            nc.vector.tensor_tensor(
                                osb[:], ps[:], bias_bc[:, osl], mybir.AluOpType.add
                            )
                            nc.sync.dma_start(out_r[nt, :, osl], osb[:])

            if isinstance(repeat, str) and repeat.startswith("loop"):
                with tc.For_i(0, int(repeat[4:]), 1):
                    rep_body()
            else:
                for _rep in range(repeat):
                    rep_body()

    nc.compile()
    return nc


def shard_inputs(x, weight_base, T_tag, bias, xt_bf16=True):
    """Host-side layout transforms + sharding. Returns in_maps for 8 cores.

    x is shipped in the kernel's internal compute encoding (bf16) --
    numerically identical to casting on device, at half the DMA traffic."""
    import ml_dtypes

    wbt = np.ascontiguousarray(weight_base.T).astype(np.int8)
    tagt = np.ascontiguousarray(T_tag.T.astype(np.float32))
    bias = np.ascontiguousarray(bias.astype(np.float32))
    xdt = ml_dtypes.bfloat16 if xt_bf16 else np.float32
    in_maps = []
    for c in range(N_CORES):
        xs = np.ascontiguousarray(x[c * N_SHARD:(c + 1) * N_SHARD, :].T.astype(xdt))
        in_maps.append({"xt": xs, "wbt": wbt, "tagt": tagt, "bias": bias})
    return in_maps


_CACHE = {}
WSTAT = False  # default build mode for kernel()


def _get_nc():
    if "nc" not in _CACHE:
        _CACHE["nc"] = build_core_graph(wstat=WSTAT)
    return _CACHE["nc"]


def assemble_output(concat_out, wstat=None):
    """Per-core outputs (concat along axis 0) -> full [N_FULL, D_OUT]."""
    if wstat is None:
        wstat = WSTAT
    if not wstat:
        return np.ascontiguousarray(concat_out)  # already [8*1024, 4096]
    shards = concat_out.reshape(N_CORES, D_OUT, N_SHARD)
    return np.concatenate([s.T for s in shards], axis=0)


def make_runtime(nc, donate=True):
    """Build an SPMD runtime for a compiled Bacc graph: a cached jitted
    callable that executes the per-core NEFF on the 8 axon NeuronCores via
    shard_map. Mirrors concourse.bass2jax.run_bass_via_pjrt, but keeps the
    jitted function so repeat calls don't re-trace/re-compile.

    donate=False keeps the output-seed buffers un-donated so the same
    device-resident zeros can be reused across calls (timing loops)."""
    import jax
    import concourse.mybir as mybir
    from jax.experimental.shard_map import shard_map
    from jax.sharding import Mesh, PartitionSpec
    from concourse.bass2jax import (
        _bass_exec_p,
        install_neuronx_cc_hook,
        partition_id_tensor,
    )

    install_neuronx_cc_hook()
    assert nc.dbg_addr is None
    partition_name = nc.partition_id_tensor.name if nc.partition_id_tensor else None

    in_names, out_names, out_avals = [], [], []
    for alloc in nc.m.functions[0].allocations:
        if not isinstance(alloc, mybir.MemoryLocationSet):
            continue
        name = alloc.memorylocations[0].name
        if alloc.kind == "ExternalInput":
            if name != partition_name:
                in_names.append(name)
        elif alloc.kind == "ExternalOutput":
            out_names.append(name)
            out_avals.append(
                jax.core.ShapedArray(tuple(alloc.tensor_shape), mybir.dt.np(alloc.dtype))
            )
    n_params = len(in_names)
    all_names = tuple(in_names) + tuple(out_names)
    if partition_name is not None:
        all_names = all_names + (partition_name,)

    def _body(*args):
        operands = list(args)
        if partition_name is not None:
            operands.append(partition_id_tensor())
        outs = _bass_exec_p.bind(
            *operands,
            out_avals=tuple(out_avals),
            in_names=all_names,
            out_names=tuple(out_names),
            lowering_input_output_aliases=(),
            sim_require_finite=True,
            sim_require_nnan=True,
            nc=nc,
        )
        return tuple(outs)

    devices = jax.devices()[:N_CORES]
    assert len(devices) == N_CORES
    mesh = Mesh(np.asarray(devices), ("core",))
    n_outs = len(out_names)
    sharded = jax.jit(
        shard_map(
            _body,
            mesh=mesh,
            in_specs=(PartitionSpec("core"),) * (n_params + n_outs),
            out_specs=(PartitionSpec("core"),) * n_outs,
            check_rep=False,
        ),
        donate_argnums=tuple(range(n_params, n_params + n_outs)) if donate else (),
        keep_unused=True,
    )
    return {
        "nc": nc,
        "fn": sharded,
        "mesh": mesh,
        "in_names": in_names,
        "out_names": out_names,
        "out_avals": out_avals,
    }


def get_runtime():
    if "rt" not in _CACHE:
        _CACHE["rt"] = make_runtime(_get_nc())
    return _CACHE["rt"]


def concat_inputs(in_maps, rt):
    return [
        np.concatenate([np.asarray(m[name]) for m in in_maps], axis=0)
        for name in rt["in_names"]
    ]


def zero_outputs(rt):
    return [
        np.zeros((N_CORES * a.shape[0], *a.shape[1:]), a.dtype) for a in rt["out_avals"]
    ]


def kernel(x, weight_base, T_tag, bias):
    rt = get_runtime()
    in_maps = shard_inputs(
        np.asarray(x), np.asarray(weight_base), np.asarray(T_tag), np.asarray(bias)
    )
    outs = rt["fn"](*concat_inputs(in_maps, rt), *zero_outputs(rt))
    return assemble_output(np.asarray(outs[rt["out_names"].index("out")]))
